# revision 28
# baseline (speedup 1.0000x reference)
"""Causal self-attention (b=4, s=2048, d=1024, 16 heads) on 8 trn2 NeuronCores.

Sharding: core c <- (batch b = c//2, head-half h = c%2).  Each core computes
q/k/v projections for its 8 heads over the full 2048-token sequence (exact
tensor-parallel split), runs causal attention for those heads, computes a
PARTIAL output projection (contracting only its local 512 attention-output
features against w_proj rows, producing all 1024 output channels), and the
pair exchanges/reduces partials with a pair-wise bf16 ReduceScatter that
lands the final result (each member keeps its half of the output channels)
directly in the output DRAM tensor.

Attention inner structure (per head-pair hp, query chunk q of 512):
  - scores^T tiles are [tk keys, tq queries] computed with k^T/q^T feature-
    major operands; both heads of the pair share one psum tile ([128, 2, 512])
    and a single strided exp (Act) that skips fully-masked key columns.
  - attn@v runs QUERY-MAJOR: lhsT = exp-tile slice [128 keys, 128 queries],
    rhs = v (token-major, [128 keys, 65] with a trailing ones column that
    accumulates the softmax denominator), accumulated over tk into a
    [128 q, 2*130] psum (both heads side by side).  This costs 65 PE rows
    per (head, key-tile, query-subtile) instead of ~512, cutting attn@v PE
    time roughly in half versus feature-major.
  - normalization is two per-partition tensor_scalar multiplies (denominator
    is per-query = per-partition in this orientation), producing a bf16
    [128 q, 128 f] staging tile that a PE transpose (via identity) flips to
    the feature-major [128 f, 128 q] layout the output projection needs.
  - emission interleaves projection / output-projection chains between
    attention steps so the PE stream has filler while Act (exp) runs.

All matmuls run bf16 operands with fp32 psum accumulation; softmax
statistics stay fp32; partial outproj sums are bf16-rounded before the
ReduceScatter (adds ~1e-3 rel err, well within tolerance).
"""

import numpy as np

N_HEADS = 16
B = 4
S = 2048
C = 1024
HD = C // N_HEADS            # 64
N_CORES = 8
H_LOC = N_HEADS // 2         # 8 heads per core
F_LOC = H_LOC * HD           # 512 local qkv features
P = 128                      # partitions
NCT = C // P                 # 8 contraction tiles over channels
NFT = F_LOC // P             # 4 local feature tiles (= head pairs)
NTT = S // P                 # 16 token tiles
TQ = 512                     # query-chunk width (one psum bank)
NQ = S // TQ                 # 4 query chunks
NSUB = TQ // P               # 4 query subtiles per chunk
SCALE = 1.0 / float(np.sqrt(HD))

_NC_CACHE = {}


def _merge(*streams):
    """Interleave emission streams (lists of thunks) uniformly."""
    tagged = []
    for si, steps in enumerate(streams):
        n = len(steps)
        for i, th in enumerate(steps):
            tagged.append(((i + 0.5) / n, si, th))
    tagged.sort(key=lambda x: (x[0], x[1]))
    for _, _, th in tagged:
        th()


def _build_nc():
    import concourse.bacc as bacc
    import concourse.tile as tile
    from concourse import mybir
    from concourse.masks import make_identity

    dt = mybir.dt
    f32, bf16 = dt.float32, dt.bfloat16
    EXP = mybir.ActivationFunctionType.Exp
    GE = mybir.AluOpType.is_ge
    ADD = mybir.AluOpType.add
    PAIRS = [[0, 1], [2, 3], [4, 5], [6, 7]]

    nc = bacc.Bacc("TRN2", num_devices=N_CORES)

    x_t = nc.dram_tensor("x_t", [C, S], bf16, kind="ExternalInput")
    w_q = nc.dram_tensor("w_q", [C, F_LOC], bf16, kind="ExternalInput")
    w_k = nc.dram_tensor("w_k", [C, F_LOC], bf16, kind="ExternalInput")
    w_v = nc.dram_tensor("w_v", [C, F_LOC], bf16, kind="ExternalInput")
    # w_p rows = this core's local attention-output features, all 1024 cols
    w_p = nc.dram_tensor("w_p", [F_LOC, C], bf16, kind="ExternalInput")
    out = nc.dram_tensor("out", [S, F_LOC], bf16, kind="ExternalOutput")

    with tile.TileContext(nc) as tc:
        with (
            tc.tile_pool(name="persist", bufs=1) as persist,
            tc.tile_pool(name="epool", bufs=6) as epool,
            tc.tile_pool(name="npool", bufs=4) as npool,
            tc.tile_pool(name="rspool", bufs=4) as rspool,
            tc.tile_pool(name="spool", bufs=2, space="PSUM") as spool,
            tc.tile_pool(name="avpool", bufs=1, space="PSUM") as avpool,
            tc.tile_pool(name="gemm", bufs=2, space="PSUM") as gemm,
            tc.tile_pool(name="drpool", bufs=1, space="DRAM") as drpool,
        ):
            # ---- resident SBUF tensors & input loads ----
            # critical path first: w_q + x chunk 0 (feeds the first
            # projection chains), spread over the three DMA queues
            xT = [persist.tile([P, S], bf16, name=f"xT{ct}", tag=f"xT{ct}")
                  for ct in range(NCT)]
            wq_sb, wk_sb, wv_sb = [], [], []
            for wdram, dst, nm in ((w_q, wq_sb, "wq"), (w_k, wk_sb, "wk"),
                                   (w_v, wv_sb, "wv")):
                for ct in range(NCT):
                    dst.append(persist.tile([P, F_LOC], bf16, name=f"{nm}{ct}",
                                            tag=f"{nm}{ct}"))
            wp_sb = [persist.tile([P, C], bf16, name=f"wp{ct}", tag=f"wp{ct}")
                     for ct in range(NFT)]

            def load_w(dst, wdram, ct, eng):
                eng.dma_start(out=dst[ct],
                              in_=wdram[ct * P:(ct + 1) * P, :])

            # chunk-0 critical loads (w_q + x[:, 0:512]): split between the
            # HWDGE path (sync/scalar) and the parallel SWDGE path (gpsimd)
            for ct in range(NCT):
                eng = nc.gpsimd if ct in (1, 3, 5) else nc.sync
                load_w(wq_sb, w_q, ct, eng)
                xeng = nc.gpsimd if ct in (0, 2, 4) else nc.scalar
                xeng.dma_start(out=xT[ct][:, 0:TQ],
                               in_=x_t[ct * P:(ct + 1) * P, 0:TQ])
            for ct in range(NCT):
                load_w(wk_sb, w_k, ct, nc.sync)
            for ct in range(NCT):
                load_w(wv_sb, w_v, ct, nc.scalar)
            # x chunks 1-3: one wide DMA per ct row (4KB-run descriptors)
            for ct in range(NCT):
                (nc.sync, nc.scalar)[ct % 2].dma_start(
                    out=xT[ct][:, TQ:S], in_=x_t[ct * P:(ct + 1) * P, TQ:S])

            qT = [persist.tile([P, S], bf16, name=f"qT{ft}", tag=f"qT{ft}")
                  for ft in range(NFT)]
            kT = [persist.tile([P, S], bf16, name=f"kT{ft}", tag=f"kT{ft}")
                  for ft in range(NFT)]
            # v token-major with a ones column per head: [token, head, 65]
            v_sb = [persist.tile([P, H_LOC, HD + 1], bf16, name=f"v{tt}",
                                 tag=f"v{tt}")
                    for tt in range(NTT)]
            for tt in range(NTT):
                nc.vector.memset(v_sb[tt][:, :, HD:HD + 1], 1.0)

            # multiply-masks for the 4 diagonal-tile offsets (two heads wide)
            masks = []
            for m in range(NSUB):
                mk = persist.tile([P, 2, TQ], bf16, name=f"mask{m}",
                                  tag=f"mask{m}")
                nc.gpsimd.memset(mk, 1.0)
                nc.gpsimd.affine_select(
                    out=mk, in_=mk, compare_op=GE, fill=0.0,
                    base=-P * m, pattern=[[0, 2], [1, TQ]],
                    channel_multiplier=-1)
                masks.append(mk)

            identf = persist.tile([P, P], f32, name="identf", tag="identf")
            make_identity(nc, identf)

            # wp loads deferred past the mask setup so the Pool queue serves
            # the attention masks first (wp is not needed until outproj(0))
            for ct in range(NFT):
                nc.gpsimd.dma_start(out=wp_sb[ct],
                                    in_=w_p[ct * P:(ct + 1) * P, :])

            # transposed (feature-major) normalized attention outputs
            aoT = {}
            for q in range(NQ):
                for hp in range(NFT):
                    for j in range(NSUB):
                        aoT[(q, hp, j)] = persist.tile(
                            [P, P], bf16, name=f"aoT_{q}_{hp}_{j}",
                            tag=f"aoT_{q}_{hp}_{j}")

            # DRAM bounce buffers for the pair-wise ReduceScatter
            rs_in = [drpool.tile([2, TQ, F_LOC], bf16, name=f"rs_in{q}",
                                 tag=f"rs_in{q}") for q in range(NQ)]
            rs_out = [drpool.tile([TQ, F_LOC], bf16, name=f"rs_out{q}",
                                  tag=f"rs_out{q}") for q in range(NQ)]

            # ---------------- emission units ----------------

            def proj_steps(q):
                """12 projection chains for token chunk q (emission thunks)."""
                qs = slice(q * TQ, (q + 1) * TQ)
                steps = []

                def qk_chain(dstT, w_sb, ft, nm):
                    def th():
                        fs = slice(ft * P, (ft + 1) * P)
                        ps = gemm.tile([P, TQ], f32, name=f"ps_{nm}{ft}_{q}",
                                       tag="gm")
                        for ct in range(NCT):
                            nc.tensor.matmul(
                                ps, lhsT=w_sb[ct][:, fs], rhs=xT[ct][:, qs],
                                start=(ct == 0), stop=(ct == NCT - 1))
                        nc.vector.tensor_copy(dstT[ft][:, qs], ps)
                    return th

                def v_chain(tt):
                    def th():
                        ts_ = slice(tt * P, (tt + 1) * P)
                        ps = gemm.tile([P, TQ], f32, name=f"ps_v{tt}",
                                       tag="gm")
                        for ct in range(NCT):
                            nc.tensor.matmul(
                                ps[:, 0:F_LOC], lhsT=xT[ct][:, ts_],
                                rhs=wv_sb[ct][:],
                                start=(ct == 0), stop=(ct == NCT - 1))
                        nc.vector.tensor_copy(
                            v_sb[tt][:, :, 0:HD],
                            ps[:, 0:F_LOC].rearrange("p (h d) -> p h d",
                                                     h=H_LOC))
                    return th

                # v first (attention consumes k/v of ALL chunks; q only own)
                for tt in range(q * NSUB, (q + 1) * NSUB):
                    steps.append(v_chain(tt))
                for ft in range(NFT):
                    steps.append(qk_chain(kT, wk_sb, ft, "k"))
                for ft in range(NFT):
                    steps.append(qk_chain(qT, wq_sb, ft, "q"))
                return steps

            def attn_steps(q, hp):
                """Attention for (chunk q, head-pair hp): ntk tk-steps; the
                last NSUB steps also emit the norm+transpose of a subtile."""
                ntk = (q + 1) * NSUB
                steps = []
                av_tiles = {}

                def tk_step(tk):
                    def th():
                        m = max(0, tk - q * NSUB)
                        c0 = P * m
                        ks = slice(tk * P, (tk + 1) * P)
                        qsm = slice(q * TQ + c0, (q + 1) * TQ)
                        s = spool.tile([P, 2, TQ], f32,
                                       name=f"s_{q}_{hp}_{tk}", tag="sc")
                        nc.tensor.matmul(s[:, 0, c0:TQ],
                                         lhsT=kT[hp][0:HD, ks],
                                         rhs=qT[hp][0:HD, qsm],
                                         start=True, stop=True)
                        nc.tensor.matmul(s[:, 1, c0:TQ],
                                         lhsT=kT[hp][HD:P, ks],
                                         rhs=qT[hp][HD:P, qsm],
                                         start=True, stop=True)
                        e = epool.tile([P, 2, TQ], bf16,
                                       name=f"e_{q}_{hp}_{tk}", tag="e")
                        nc.scalar.activation(out=e[:, :, c0:TQ],
                                             in_=s[:, :, c0:TQ],
                                             func=EXP, scale=SCALE)
                        if m > 0 or tk == q * NSUB:
                            nc.vector.tensor_mul(e[:, :, c0:TQ],
                                                 e[:, :, c0:TQ],
                                                 masks[m][:, :, c0:TQ])
                        for j in range(NSUB):
                            if j < m:
                                continue
                            if j % 2 == 0 and j // 2 not in av_tiles:
                                av_tiles[j // 2] = avpool.tile(
                                    [P, 388], f32, name=f"av_{q}_{hp}_{j//2}",
                                    tag=f"av{j // 2}")
                            avt = av_tiles[j // 2]
                            base = (j % 2) * 130
                            # one psum accumulation group per BANK: start
                            # only on the first matmul into the bank (hw
                            # marks the whole 2KB region pending-zero; each
                            # region's first write then overwrites), stop
                            # only on the chronologically last one.
                            first = (tk == 0 and j % 2 == 0)
                            last = (j % 2 == 1 and tk == q * NSUB + j)
                            nc.tensor.matmul(
                                avt[:, base:base + 65],
                                lhsT=e[:, 0, j * P:(j + 1) * P],
                                rhs=v_sb[tk][:, 2 * hp, :],
                                start=first, stop=False)
                            nc.tensor.matmul(
                                avt[:, base + 65:base + 130],
                                lhsT=e[:, 1, j * P:(j + 1) * P],
                                rhs=v_sb[tk][:, 2 * hp + 1, :],
                                start=False, stop=last)
                        j_done = tk - q * NSUB
                        if j_done >= 0 and j_done % 2 == 1:
                            # the av bank's accumulation group stopped at
                            # this tk: its values may now be read and its
                            # [260:388] region used for the two transposes
                            norm_step(j_done - 1)
                            norm_step(j_done)
                            transpose_step(j_done - 1)
                            transpose_step(j_done)
                    return th

                stages = {}

                def norm_step(j):
                    avt = av_tiles[j // 2]
                    base = (j % 2) * 130
                    rec = npool.tile([P, 2], f32, name=f"rec_{q}_{hp}_{j}",
                                     tag="rec")
                    nc.vector.reciprocal(rec[:, 0:1],
                                         avt[:, base + 64:base + 65])
                    nc.vector.reciprocal(rec[:, 1:2],
                                         avt[:, base + 129:base + 130])
                    stage = npool.tile([P, P], f32, name=f"st_{q}_{hp}_{j}",
                                       tag=f"stage{j % 2}")
                    nc.vector.tensor_scalar_mul(
                        stage[:, 0:HD], avt[:, base:base + HD], rec[:, 0:1])
                    nc.vector.tensor_scalar_mul(
                        stage[:, HD:P], avt[:, base + 65:base + 65 + HD],
                        rec[:, 1:2])
                    stages[j] = stage

                def transpose_step(j):
                    tp = av_tiles[j // 2][:, 260:388]
                    nc.tensor.transpose(out=tp, in_=stages[j],
                                        identity=identf)
                    nc.vector.tensor_copy(aoT[(q, hp, j)], tp)

                for tk in range(ntk):
                    steps.append(tk_step(tk))
                return steps

            def op_steps(q):
                """Partial output projection for chunk q (4 tt steps), then
                the ReduceScatter appended to the last step."""
                steps = []

                def tt_step(tt):
                    def th():
                        rs_stage = rspool.tile([P, 2, F_LOC], bf16,
                                               name=f"rss_{q}_{tt}",
                                               tag="rss")
                        for h2 in range(2):
                            po = gemm.tile([P, F_LOC], f32,
                                           name=f"po_{q}_{tt}_{h2}", tag="gm")
                            for ct in range(NFT):
                                nc.tensor.matmul(
                                    po,
                                    lhsT=aoT[(q, ct, tt)],
                                    rhs=wp_sb[ct][:, h2 * F_LOC:
                                                  (h2 + 1) * F_LOC],
                                    start=(ct == 0), stop=(ct == NFT - 1))
                            nc.vector.tensor_copy(rs_stage[:, h2, :], po)
                        nc.sync.dma_start(
                            out=rs_in[q].rearrange("h t c -> t h c")
                            [tt * P:(tt + 1) * P],
                            in_=rs_stage)
                        if tt == NSUB - 1:
                            nc.gpsimd.collective_compute(
                                "ReduceScatter",
                                ADD,
                                replica_groups=PAIRS,
                                ins=[rs_in[q][:].opt()],
                                outs=[rs_out[q][:].opt()],
                            )
                            nc.gpsimd.dma_start(
                                out=out[q * TQ:(q + 1) * TQ, :],
                                in_=rs_out[q][:])
                    return th

                for tt in range(NSUB):
                    steps.append(tt_step(tt))
                return steps

            # ---------------- schedule ----------------
            # W0: chunk-0 projections alone (DMA-paced startup)
            for th in proj_steps(0):
                th()
            # W1: chunk-1 projections + chunk-0 attention
            _merge(proj_steps(1),
                   attn_steps(0, 0) + attn_steps(0, 1) +
                   attn_steps(0, 2) + attn_steps(0, 3))
            # W2: chunk-2 projections + chunk-1 attention
            _merge(proj_steps(2),
                   attn_steps(1, 0) + attn_steps(1, 1) +
                   attn_steps(1, 2) + attn_steps(1, 3))
            # W3: chunk-3 projections + outproj(0)+RS(0) + attention 2.hp0-2
            _merge(proj_steps(3) + op_steps(0),
                   attn_steps(2, 0) + attn_steps(2, 1) + attn_steps(2, 2))
            # W4: outproj(1)+RS(1) + attention 3.hp0-1
            _merge(op_steps(1),
                   attn_steps(3, 0) + attn_steps(3, 1))
            # W5: outproj(2)+RS(2) + attention 2.hp3 + 3.hp2
            _merge(op_steps(2),
                   attn_steps(2, 3) + attn_steps(3, 2))
            # W6: attention 3.hp3, then outproj(3)+RS(3)
            for th in attn_steps(3, 3):
                th()
            for th in op_steps(3):
                th()

    if not nc.is_finalized():
        nc.finalize()
    return nc


def _get_nc():
    if "nc" not in _NC_CACHE:
        _NC_CACHE["nc"] = _build_nc()
    return _NC_CACHE["nc"]


def kernel(x, w_qkv, w_proj):
    import ml_dtypes
    from concourse.bass_utils import run_bass_kernel_spmd

    bf = ml_dtypes.bfloat16
    x = np.asarray(x, dtype=np.float32)
    w_qkv = np.asarray(w_qkv, dtype=np.float32)
    w_proj = np.asarray(w_proj, dtype=np.float32)

    xT = np.ascontiguousarray(x.transpose(0, 2, 1)).astype(bf)  # [B, C, S]
    in_maps = []
    for c in range(N_CORES):
        bi, hi = c // 2, c % 2
        fs = slice(F_LOC * hi, F_LOC * (hi + 1))
        in_maps.append({
            "x_t": xT[bi],
            "w_q": np.ascontiguousarray(w_qkv[:, 0 * C:1 * C][:, fs]).astype(bf),
            "w_k": np.ascontiguousarray(w_qkv[:, 1 * C:2 * C][:, fs]).astype(bf),
            "w_v": np.ascontiguousarray(w_qkv[:, 2 * C:3 * C][:, fs]).astype(bf),
            "w_p": np.ascontiguousarray(w_proj[fs, :]).astype(bf),
        })

    res = run_bass_kernel_spmd(_get_nc(), in_maps,
                               core_ids=list(range(N_CORES)))
    _NC_CACHE["last_res"] = res

    # RS member h keeps output-channel half h; concat per batch pair
    out = np.stack([
        np.concatenate([np.asarray(res.results[2 * bi]["out"]),
                        np.asarray(res.results[2 * bi + 1]["out"])], axis=1)
        for bi in range(B)]).astype(np.float32)
    return out


# revision 29
# speedup vs baseline: 1.0050x; 1.0050x over previous
"""Causal self-attention (b=4, s=2048, d=1024, 16 heads) on 8 trn2 NeuronCores.

Sharding: core c <- (batch b = c//2, head-half h = c%2).  Each core computes
q/k/v projections for its 8 heads over the full 2048-token sequence (exact
tensor-parallel split), runs causal attention for those heads, computes a
PARTIAL output projection (contracting only its local 512 attention-output
features against w_proj rows, producing all 1024 output channels), and the
pair exchanges/reduces partials with a pair-wise bf16 ReduceScatter that
lands the final result (each member keeps its half of the output channels)
directly in the output DRAM tensor.

Attention inner structure (per head-pair hp, query chunk q of 512):
  - scores^T tiles are [tk keys, tq queries] computed with k^T/q^T feature-
    major operands; both heads of the pair share one psum tile ([128, 2, 512])
    and a single strided exp (Act) that skips fully-masked key columns.
  - attn@v runs QUERY-MAJOR: lhsT = exp-tile slice [128 keys, 128 queries],
    rhs = v (token-major, [128 keys, 65] with a trailing ones column that
    accumulates the softmax denominator), accumulated over tk into a
    [128 q, 2*130] psum (both heads side by side).  This costs 65 PE rows
    per (head, key-tile, query-subtile) instead of ~512, cutting attn@v PE
    time roughly in half versus feature-major.
  - normalization is two per-partition tensor_scalar multiplies (denominator
    is per-query = per-partition in this orientation), producing a bf16
    [128 q, 128 f] staging tile that a PE transpose (via identity) flips to
    the feature-major [128 f, 128 q] layout the output projection needs.
  - emission interleaves projection / output-projection chains between
    attention steps so the PE stream has filler while Act (exp) runs.

All matmuls run bf16 operands with fp32 psum accumulation; softmax
statistics stay fp32; partial outproj sums are bf16-rounded before the
ReduceScatter (adds ~1e-3 rel err, well within tolerance).
"""

import numpy as np

N_HEADS = 16
B = 4
S = 2048
C = 1024
HD = C // N_HEADS            # 64
N_CORES = 8
H_LOC = N_HEADS // 2         # 8 heads per core
F_LOC = H_LOC * HD           # 512 local qkv features
P = 128                      # partitions
NCT = C // P                 # 8 contraction tiles over channels
NFT = F_LOC // P             # 4 local feature tiles (= head pairs)
NTT = S // P                 # 16 token tiles
TQ = 512                     # query-chunk width (one psum bank)
NQ = S // TQ                 # 4 query chunks
NSUB = TQ // P               # 4 query subtiles per chunk
SCALE = 1.0 / float(np.sqrt(HD))

_NC_CACHE = {}


def _merge(*streams):
    """Interleave emission streams (lists of thunks) uniformly."""
    tagged = []
    for si, steps in enumerate(streams):
        n = len(steps)
        for i, th in enumerate(steps):
            tagged.append(((i + 0.5) / n, si, th))
    tagged.sort(key=lambda x: (x[0], x[1]))
    for _, _, th in tagged:
        th()


def _build_nc():
    import concourse.bacc as bacc
    import concourse.tile as tile
    from concourse import mybir
    from concourse.masks import make_identity

    dt = mybir.dt
    f32, bf16 = dt.float32, dt.bfloat16
    EXP = mybir.ActivationFunctionType.Exp
    GE = mybir.AluOpType.is_ge
    ADD = mybir.AluOpType.add
    PAIRS = [[0, 1], [2, 3], [4, 5], [6, 7]]

    nc = bacc.Bacc("TRN2", num_devices=N_CORES)

    x_t = nc.dram_tensor("x_t", [C, S], bf16, kind="ExternalInput")
    w_q = nc.dram_tensor("w_q", [C, F_LOC], bf16, kind="ExternalInput")
    w_k = nc.dram_tensor("w_k", [C, F_LOC], bf16, kind="ExternalInput")
    w_v = nc.dram_tensor("w_v", [C, F_LOC], bf16, kind="ExternalInput")
    # w_p rows = this core's local attention-output features, all 1024 cols
    w_p = nc.dram_tensor("w_p", [F_LOC, C], bf16, kind="ExternalInput")
    out = nc.dram_tensor("out", [S, F_LOC], bf16, kind="ExternalOutput")

    with tile.TileContext(nc) as tc:
        with (
            tc.tile_pool(name="persist", bufs=1) as persist,
            tc.tile_pool(name="epool", bufs=6) as epool,
            tc.tile_pool(name="npool", bufs=4) as npool,
            tc.tile_pool(name="rspool", bufs=4) as rspool,
            tc.tile_pool(name="spool", bufs=2, space="PSUM") as spool,
            tc.tile_pool(name="avpool", bufs=1, space="PSUM") as avpool,
            tc.tile_pool(name="gemm", bufs=2, space="PSUM") as gemm,
            tc.tile_pool(name="drpool", bufs=1, space="DRAM") as drpool,
        ):
            # ---- resident SBUF tensors & input loads ----
            # critical path first: w_q + x chunk 0 (feeds the first
            # projection chains), spread over the three DMA queues
            xT = [persist.tile([P, S], bf16, name=f"xT{ct}", tag=f"xT{ct}")
                  for ct in range(NCT)]
            wq_sb, wk_sb, wv_sb = [], [], []
            for wdram, dst, nm in ((w_q, wq_sb, "wq"), (w_k, wk_sb, "wk"),
                                   (w_v, wv_sb, "wv")):
                for ct in range(NCT):
                    dst.append(persist.tile([P, F_LOC], bf16, name=f"{nm}{ct}",
                                            tag=f"{nm}{ct}"))
            wp_sb = [persist.tile([P, C], bf16, name=f"wp{ct}", tag=f"wp{ct}")
                     for ct in range(NFT)]

            def load_w(dst, wdram, ct, eng):
                eng.dma_start(out=dst[ct],
                              in_=wdram[ct * P:(ct + 1) * P, :])

            # chunk-0 critical loads (w_q + x[:, 0:512]): split between the
            # HWDGE path (sync/scalar) and the parallel SWDGE path (gpsimd)
            for ct in range(NCT):
                eng = nc.gpsimd if ct in (1, 3, 5) else nc.sync
                load_w(wq_sb, w_q, ct, eng)
                xeng = nc.gpsimd if ct in (0, 2, 4) else nc.scalar
                xeng.dma_start(out=xT[ct][:, 0:TQ],
                               in_=x_t[ct * P:(ct + 1) * P, 0:TQ])
            for ct in range(NCT):
                load_w(wk_sb, w_k, ct, nc.sync)
            for ct in range(NCT):
                load_w(wv_sb, w_v, ct, nc.scalar)
            # x chunks 1-3: one wide DMA per ct row (4KB-run descriptors)
            for ct in range(NCT):
                (nc.sync, nc.scalar)[ct % 2].dma_start(
                    out=xT[ct][:, TQ:S], in_=x_t[ct * P:(ct + 1) * P, TQ:S])

            qT = [persist.tile([P, S], bf16, name=f"qT{ft}", tag=f"qT{ft}")
                  for ft in range(NFT)]
            kT = [persist.tile([P, S], bf16, name=f"kT{ft}", tag=f"kT{ft}")
                  for ft in range(NFT)]
            # v token-major with a ones column per head: [token, head, 65]
            v_sb = [persist.tile([P, H_LOC, HD + 1], bf16, name=f"v{tt}",
                                 tag=f"v{tt}")
                    for tt in range(NTT)]
            for tt in range(NTT):
                nc.vector.memset(v_sb[tt][:, :, HD:HD + 1], 1.0)

            # multiply-masks for the 4 diagonal-tile offsets (two heads wide)
            masks = []
            for m in range(NSUB):
                mk = persist.tile([P, 2, TQ], bf16, name=f"mask{m}",
                                  tag=f"mask{m}")
                nc.gpsimd.memset(mk, 1.0)
                nc.gpsimd.affine_select(
                    out=mk, in_=mk, compare_op=GE, fill=0.0,
                    base=-P * m, pattern=[[0, 2], [1, TQ]],
                    channel_multiplier=-1)
                masks.append(mk)

            identf = persist.tile([P, P], f32, name="identf", tag="identf")
            make_identity(nc, identf)

            # wp loads deferred past the mask setup so the Pool queue serves
            # the attention masks first (wp is not needed until outproj(0))
            for ct in range(NFT):
                nc.gpsimd.dma_start(out=wp_sb[ct],
                                    in_=w_p[ct * P:(ct + 1) * P, :])

            # transposed (feature-major) normalized attention outputs
            aoT = {}
            for q in range(NQ):
                for hp in range(NFT):
                    for j in range(NSUB):
                        aoT[(q, hp, j)] = persist.tile(
                            [P, P], bf16, name=f"aoT_{q}_{hp}_{j}",
                            tag=f"aoT_{q}_{hp}_{j}")

            # DRAM bounce buffers for the pair-wise ReduceScatter
            rs_in = [drpool.tile([2, TQ, F_LOC], bf16, name=f"rs_in{q}",
                                 tag=f"rs_in{q}") for q in range(NQ)]
            rs_out = [drpool.tile([TQ, F_LOC], bf16, name=f"rs_out{q}",
                                  tag=f"rs_out{q}") for q in range(NQ)]

            # ---------------- emission units ----------------

            def proj_steps(q):
                """12 projection chains for token chunk q (emission thunks)."""
                qs = slice(q * TQ, (q + 1) * TQ)
                steps = []

                def qk_chain(dstT, w_sb, ft, nm):
                    def th():
                        fs = slice(ft * P, (ft + 1) * P)
                        ps = gemm.tile([P, TQ], f32, name=f"ps_{nm}{ft}_{q}",
                                       tag="gm")
                        for ct in range(NCT):
                            nc.tensor.matmul(
                                ps, lhsT=w_sb[ct][:, fs], rhs=xT[ct][:, qs],
                                start=(ct == 0), stop=(ct == NCT - 1))
                        nc.vector.tensor_copy(dstT[ft][:, qs], ps)
                    return th

                def v_chain(tt):
                    def th():
                        ts_ = slice(tt * P, (tt + 1) * P)
                        ps = gemm.tile([P, TQ], f32, name=f"ps_v{tt}",
                                       tag="gm")
                        for ct in range(NCT):
                            nc.tensor.matmul(
                                ps[:, 0:F_LOC], lhsT=xT[ct][:, ts_],
                                rhs=wv_sb[ct][:],
                                start=(ct == 0), stop=(ct == NCT - 1))
                        nc.vector.tensor_copy(
                            v_sb[tt][:, :, 0:HD],
                            ps[:, 0:F_LOC].rearrange("p (h d) -> p h d",
                                                     h=H_LOC))
                    return th

                # v first (attention consumes k/v of ALL chunks; q only own)
                for tt in range(q * NSUB, (q + 1) * NSUB):
                    steps.append(v_chain(tt))
                for ft in range(NFT):
                    steps.append(qk_chain(kT, wk_sb, ft, "k"))
                for ft in range(NFT):
                    steps.append(qk_chain(qT, wq_sb, ft, "q"))
                return steps

            def attn_steps(q, hp):
                """Attention for (chunk q, head-pair hp): ntk tk-steps; the
                last NSUB steps also emit the norm+transpose of a subtile."""
                ntk = (q + 1) * NSUB
                steps = []
                av_tiles = {}

                def tk_step(tk):
                    def th():
                        m = max(0, tk - q * NSUB)
                        c0 = P * m
                        ks = slice(tk * P, (tk + 1) * P)
                        qsm = slice(q * TQ + c0, (q + 1) * TQ)
                        s = spool.tile([P, 2, TQ], f32,
                                       name=f"s_{q}_{hp}_{tk}", tag="sc")
                        nc.tensor.matmul(s[:, 0, c0:TQ],
                                         lhsT=kT[hp][0:HD, ks],
                                         rhs=qT[hp][0:HD, qsm],
                                         start=True, stop=True)
                        nc.tensor.matmul(s[:, 1, c0:TQ],
                                         lhsT=kT[hp][HD:P, ks],
                                         rhs=qT[hp][HD:P, qsm],
                                         start=True, stop=True)
                        e = epool.tile([P, 2, TQ], bf16,
                                       name=f"e_{q}_{hp}_{tk}", tag="e")
                        nc.scalar.activation(out=e[:, :, c0:TQ],
                                             in_=s[:, :, c0:TQ],
                                             func=EXP, scale=SCALE)
                        if m > 0 or tk == q * NSUB:
                            nc.vector.tensor_mul(e[:, :, c0:TQ],
                                                 e[:, :, c0:TQ],
                                                 masks[m][:, :, c0:TQ])
                        for j in range(NSUB):
                            if j < m:
                                continue
                            if j % 2 == 0 and j // 2 not in av_tiles:
                                av_tiles[j // 2] = avpool.tile(
                                    [P, 388], f32, name=f"av_{q}_{hp}_{j//2}",
                                    tag=f"av{j // 2}")
                            avt = av_tiles[j // 2]
                            base = (j % 2) * 130
                            # one psum accumulation group per BANK: start
                            # only on the first matmul into the bank (hw
                            # marks the whole 2KB region pending-zero; each
                            # region's first write then overwrites), stop
                            # only on the chronologically last one.
                            first = (tk == 0 and j % 2 == 0)
                            last = (j % 2 == 1 and tk == q * NSUB + j)
                            nc.tensor.matmul(
                                avt[:, base:base + 65],
                                lhsT=e[:, 0, j * P:(j + 1) * P],
                                rhs=v_sb[tk][:, 2 * hp, :],
                                start=first, stop=False)
                            nc.tensor.matmul(
                                avt[:, base + 65:base + 130],
                                lhsT=e[:, 1, j * P:(j + 1) * P],
                                rhs=v_sb[tk][:, 2 * hp + 1, :],
                                start=False, stop=last)
                        j_done = tk - q * NSUB
                        if j_done >= 0 and j_done % 2 == 1:
                            # the av bank's accumulation group stopped at
                            # this tk: its values may now be read and its
                            # [260:388] region used for the two transposes
                            norm_step(j_done - 1)
                            norm_step(j_done)
                            transpose_step(j_done - 1)
                            transpose_step(j_done)
                    return th

                stages = {}

                def norm_step(j):
                    avt = av_tiles[j // 2]
                    base = (j % 2) * 130
                    rec = npool.tile([P, 2], f32, name=f"rec_{q}_{hp}_{j}",
                                     tag="rec")
                    nc.vector.reciprocal(rec[:, 0:1],
                                         avt[:, base + 64:base + 65])
                    nc.vector.reciprocal(rec[:, 1:2],
                                         avt[:, base + 129:base + 130])
                    stage = npool.tile([P, P], f32, name=f"st_{q}_{hp}_{j}",
                                       tag=f"stage{j % 2}")
                    nc.vector.tensor_scalar_mul(
                        stage[:, 0:HD], avt[:, base:base + HD], rec[:, 0:1])
                    nc.vector.tensor_scalar_mul(
                        stage[:, HD:P], avt[:, base + 65:base + 65 + HD],
                        rec[:, 1:2])
                    stages[j] = stage

                def transpose_step(j):
                    tp = av_tiles[j // 2][:, 260:388]
                    nc.tensor.transpose(out=tp, in_=stages[j],
                                        identity=identf)
                    nc.vector.tensor_copy(aoT[(q, hp, j)], tp)

                for tk in range(ntk):
                    steps.append(tk_step(tk))
                return steps

            def op_steps(q):
                """Partial output projection for chunk q (4 tt steps), then
                the ReduceScatter appended to the last step."""
                steps = []

                def tt_step(tt):
                    def th():
                        rs_stage = rspool.tile([P, 2, F_LOC], bf16,
                                               name=f"rss_{q}_{tt}",
                                               tag="rss")
                        for h2 in range(2):
                            po = gemm.tile([P, F_LOC], f32,
                                           name=f"po_{q}_{tt}_{h2}", tag="gm")
                            for ct in range(NFT):
                                nc.tensor.matmul(
                                    po,
                                    lhsT=aoT[(q, ct, tt)],
                                    rhs=wp_sb[ct][:, h2 * F_LOC:
                                                  (h2 + 1) * F_LOC],
                                    start=(ct == 0), stop=(ct == NFT - 1))
                            nc.vector.tensor_copy(rs_stage[:, h2, :], po)
                        nc.sync.dma_start(
                            out=rs_in[q].rearrange("h t c -> t h c")
                            [tt * P:(tt + 1) * P],
                            in_=rs_stage)
                        if tt == NSUB - 1:
                            nc.gpsimd.collective_compute(
                                "ReduceScatter",
                                ADD,
                                replica_groups=PAIRS,
                                ins=[rs_in[q][:].opt()],
                                outs=[rs_out[q][:].opt()],
                            )
                            nc.gpsimd.dma_start(
                                out=out[q * TQ:(q + 1) * TQ, :],
                                in_=rs_out[q][:])
                    return th

                for tt in range(NSUB):
                    steps.append(tt_step(tt))
                return steps

            # ---------------- schedule ----------------
            # W0: chunk-0 projections alone (DMA-paced startup)
            for th in proj_steps(0):
                th()
            # W1: chunk-1 projections + chunk-0 attention
            _merge(proj_steps(1),
                   attn_steps(0, 0) + attn_steps(0, 1) +
                   attn_steps(0, 2) + attn_steps(0, 3))
            # W2: chunk-2 projections + chunk-1 attention
            _merge(proj_steps(2),
                   attn_steps(1, 0) + attn_steps(1, 1) +
                   attn_steps(1, 2) + attn_steps(1, 3))
            # W3: chunk-3 projections + outproj(0)+RS(0) + attention 2.hp0-2
            _merge(proj_steps(3) + op_steps(0),
                   attn_steps(2, 0) + attn_steps(2, 1) + attn_steps(2, 2))
            # W4: outproj(1)+RS(1) + attention 2.hp3 + 3.hp0
            _merge(op_steps(1),
                   attn_steps(2, 3) + attn_steps(3, 0))
            # W5: outproj(2)+RS(2) + attention 3.hp1-2
            _merge(op_steps(2),
                   attn_steps(3, 1) + attn_steps(3, 2))
            # W6: attention 3.hp3, then outproj(3)+RS(3)
            for th in attn_steps(3, 3):
                th()
            for th in op_steps(3):
                th()

    if not nc.is_finalized():
        nc.finalize()
    return nc


def _get_nc():
    if "nc" not in _NC_CACHE:
        _NC_CACHE["nc"] = _build_nc()
    return _NC_CACHE["nc"]


def kernel(x, w_qkv, w_proj):
    import ml_dtypes
    from concourse.bass_utils import run_bass_kernel_spmd

    bf = ml_dtypes.bfloat16
    x = np.asarray(x, dtype=np.float32)
    w_qkv = np.asarray(w_qkv, dtype=np.float32)
    w_proj = np.asarray(w_proj, dtype=np.float32)

    xT = np.ascontiguousarray(x.transpose(0, 2, 1)).astype(bf)  # [B, C, S]
    in_maps = []
    for c in range(N_CORES):
        bi, hi = c // 2, c % 2
        fs = slice(F_LOC * hi, F_LOC * (hi + 1))
        in_maps.append({
            "x_t": xT[bi],
            "w_q": np.ascontiguousarray(w_qkv[:, 0 * C:1 * C][:, fs]).astype(bf),
            "w_k": np.ascontiguousarray(w_qkv[:, 1 * C:2 * C][:, fs]).astype(bf),
            "w_v": np.ascontiguousarray(w_qkv[:, 2 * C:3 * C][:, fs]).astype(bf),
            "w_p": np.ascontiguousarray(w_proj[fs, :]).astype(bf),
        })

    res = run_bass_kernel_spmd(_get_nc(), in_maps,
                               core_ids=list(range(N_CORES)))
    _NC_CACHE["last_res"] = res

    # RS member h keeps output-channel half h; concat per batch pair
    out = np.stack([
        np.concatenate([np.asarray(res.results[2 * bi]["out"]),
                        np.asarray(res.results[2 * bi + 1]["out"])], axis=1)
        for bi in range(B)]).astype(np.float32)
    return out
